# revision 12
# baseline (speedup 1.0000x reference)
"""MoE FFN (dMoE) on 8 Trainium2 NeuronCores, expert-parallel.

Strategy (per sharding hint): one expert per core. The host performs the
cheap, bandwidth-trivial routing math (LayerNorm, router logits, top-2,
capacity-packed dispatch) exactly as the fp32 reference does, packs the
[E, C, D] buffer, and ships expert e's packed tokens + weights to core e.
Each core runs the compute-dominant grouped SwiGLU FFN
  gu = xb @ w12.T ; h = silu(g) * u ; y = h @ w3.T
as a Bass/Tile kernel in bf16 with fp32 PSUM accumulation, laid out so no
on-device transposes are needed (everything is produced f-major /
d-major "transposed" already). Host applies the gate weights and
scatter-adds partial outputs back to token order (the "combine").
"""

import math
import os
import sys

sys.path.insert(0, "/opt/trn_rl_repo")

import ml_dtypes
import numpy as np

import concourse.bass as bass
import concourse.bacc as bacc
import concourse.mybir as mybir
import concourse.tile as tile
from concourse.bass import ds
from concourse.bass_utils import run_bass_kernel_spmd

D = 1024          # d_model
F = 4096          # d_ff
E = 8             # experts == cores
TOPK = 2
T = 2 * 2048      # tokens
C = max(1, math.ceil(T * TOPK * 1.25 / E))  # 1280 per-expert capacity
CLAMP = 1e4
LN_EPS = 1e-5

BF16 = mybir.dt.bfloat16
FP32 = mybir.dt.float32

KD = D // 128     # 8  d-chunks (contraction, phase 1)
MF = F // 128     # 32 f-tiles per half (g / u)
KF = F // 128     # 32 f-chunks (contraction, phase 2)
C_RUNS = [(0, 512), (512, 512), (1024, 256)]   # c free-dim runs
FSC = 256         # f superchunk per w12 DMA (2 f-tiles)

_CACHED = {}


def build_nc():
    nc = bacc.Bacc()
    xbT = nc.declare_dram_parameter("xbT", [D, C], BF16, isOutput=False)
    w12T = nc.declare_dram_parameter("w12T", [D, 2 * F], BF16, isOutput=False)
    w3T = nc.declare_dram_parameter("w3T", [F, D], BF16, isOutput=False)
    yT = nc.declare_dram_parameter("yT", [D, C], FP32, isOutput=True)

    xbT_r = xbT.rearrange("(k p) c -> p k c", p=128)      # [128, KD, C]
    w12T_r = w12T.rearrange("(k p) f -> p k f", p=128)    # [128, KD, 2F]
    w3T_r = w3T.rearrange("(k p) d -> p k d", p=128)      # [128, KF, D]
    yT_r = yT.rearrange("(m p) c -> m p c", p=128)        # [8, 128, C]

    with tile.TileContext(nc) as tc:
        with (
            tc.tile_pool(name="persist", bufs=1) as persist,
            tc.tile_pool(name="w12", bufs=2) as w12_pool,
            tc.tile_pool(name="w3", bufs=2) as w3_pool,
            tc.tile_pool(name="act", bufs=3) as act_pool,
            tc.tile_pool(name="out", bufs=8) as out_pool,
        ):
            xb_sb = persist.tile([128, KD, C], BF16)
            nc.sync.dma_start(xb_sb[:], xbT_r)
            hT = persist.tile([128, KF, C], BF16)

            # ---------------- phase 1: guT = w12T.T-chunks @ xbT, silu ----
            with tc.tile_pool(name="ps1", bufs=1, space="PSUM") as ps1:
                for sc in range(F // FSC):           # 16 superchunks
                    wg = w12_pool.tile([128, KD, FSC], BF16, tag="wg")
                    wu = w12_pool.tile([128, KD, FSC], BF16, tag="wu")
                    nc.sync.dma_start(wg[:], w12T_r[:, :, ds(sc * FSC, FSC)])
                    nc.sync.dma_start(wu[:], w12T_r[:, :, ds(F + sc * FSC, FSC)])
                    for mj in range(FSC // 128):
                        m = sc * (FSC // 128) + mj   # f-tile index 0..31
                        g_ps = [ps1.tile([128, cn], FP32, tag=f"g{i}", name=f"g_ps{i}")
                                for i, (_, cn) in enumerate(C_RUNS)]
                        u_ps = [ps1.tile([128, cn], FP32, tag=f"u{i}", name=f"u_ps{i}")
                                for i, (_, cn) in enumerate(C_RUNS)]
                        for k in range(KD):
                            for i, (c0, cn) in enumerate(C_RUNS):
                                nc.tensor.matmul(
                                    g_ps[i][:],
                                    wg[:, k, ds(mj * 128, 128)],
                                    xb_sb[:, k, ds(c0, cn)],
                                    start=(k == 0), stop=(k == KD - 1))
                            for i, (c0, cn) in enumerate(C_RUNS):
                                nc.tensor.matmul(
                                    u_ps[i][:],
                                    wu[:, k, ds(mj * 128, 128)],
                                    xb_sb[:, k, ds(c0, cn)],
                                    start=(k == 0), stop=(k == KD - 1))
                        # h = (g*u) * sigmoid(g). DVE reads PSUM one operand
                        # at a time (single PSUM port) and every 2-input TT
                        # carries at most one cross-engine wait (ACT); ACT
                        # only reads DVE-written SBUF.
                        for i, (c0, cn) in enumerate(C_RUNS):
                            g_sb = act_pool.tile([128, 512], FP32, tag="g_sb")
                            u_sb = act_pool.tile([128, 512], FP32, tag="u_sb")
                            gu_t = act_pool.tile([128, 512], FP32, tag="gu_t")
                            sig = act_pool.tile([128, 512], FP32, tag="sig")
                            sig2 = act_pool.tile([128, 512], FP32, tag="sig2")
                            nc.vector.tensor_copy(g_sb[:, :cn], g_ps[i][:])
                            nc.vector.tensor_copy(u_sb[:, :cn], u_ps[i][:])
                            nc.scalar.activation(
                                sig[:, :cn], g_sb[:, :cn],
                                mybir.ActivationFunctionType.Sigmoid)
                            nc.vector.tensor_mul(
                                gu_t[:, :cn], g_sb[:, :cn], u_sb[:, :cn])
                            # bounce sig through DVE so the final TT only
                            # joins DVE-produced tiles (1-wait limit on TT)
                            nc.vector.tensor_copy(sig2[:, :cn], sig[:, :cn])
                            nc.vector.tensor_mul(
                                hT[:, m, ds(c0, cn)], gu_t[:, :cn],
                                sig2[:, :cn])

            # ---------------- phase 2: yT = w3T-chunks.T @ hT --------------
            with tc.tile_pool(name="ps2", bufs=2, space="PSUM") as ps2:
                for md in range(D // 128):           # 8 output d-tiles
                    w3t = w3_pool.tile([128, KF, 128], BF16, tag="w3t")
                    nc.sync.dma_start(w3t[:], w3T_r[:, :, ds(md * 128, 128)])
                    y_ps = [ps2.tile([128, cn], FP32, tag=f"y{i}", name=f"y_ps{i}")
                            for i, (_, cn) in enumerate(C_RUNS)]
                    for k in range(KF):
                        for i, (c0, cn) in enumerate(C_RUNS):
                            nc.tensor.matmul(
                                y_ps[i][:],
                                w3t[:, k, :],
                                hT[:, k, ds(c0, cn)],
                                start=(k == 0), stop=(k == KF - 1))
                    y_sb = out_pool.tile([128, C], FP32, tag="ysb")
                    for i, (c0, cn) in enumerate(C_RUNS):
                        nc.vector.tensor_copy(y_sb[:, ds(c0, cn)], y_ps[i][:])
                    nc.sync.dma_start(yT_r[md], y_sb[:])
    nc.finalize()
    return nc


def _route(x, ln_gamma, ln_beta, router_w):
    """Exact fp32 replica of the reference routing math (numpy)."""
    xf = x.reshape(T, D).astype(np.float32)
    mu = xf.mean(axis=-1, keepdims=True, dtype=np.float32)
    var = np.mean((xf - mu) ** 2, axis=-1, keepdims=True, dtype=np.float32)
    xn = ((xf - mu) * (1.0 / np.sqrt(var + LN_EPS))) * ln_gamma + ln_beta
    xn = xn.astype(np.float32)
    logits = np.clip(xn @ router_w.T.astype(np.float32), -CLAMP, CLAMP)
    # top-2 (ties -> lowest index, matching jax.lax.top_k)
    i1 = np.argmax(logits, axis=-1)
    v1 = np.take_along_axis(logits, i1[:, None], axis=-1)[:, 0]
    masked = logits.copy()
    np.put_along_axis(masked, i1[:, None], -np.inf, axis=-1)
    i2 = np.argmax(masked, axis=-1)
    v2 = np.take_along_axis(masked, i2[:, None], axis=-1)[:, 0]
    top_v = np.stack([v1, v2], axis=-1)
    top_i = np.stack([i1, i2], axis=-1)
    m = top_v.max(axis=-1, keepdims=True)
    ev = np.exp(top_v - m)
    top_p = ev / (ev.sum(axis=-1, keepdims=True) + 1e-12)

    experts = top_i.reshape(-1)
    weights = top_p.reshape(-1).astype(np.float32)
    tokens = np.repeat(np.arange(T), TOPK)
    oh = (experts[:, None] == np.arange(E)[None, :]).astype(np.int64)
    pos = np.take_along_axis(np.cumsum(oh, axis=0) - 1, experts[:, None], 1)[:, 0]
    kept = pos < C
    slot = np.where(kept, experts * C + pos, E * C)
    return xn, experts, weights, tokens, pos, kept, slot


def kernel(x, ln_gamma, ln_beta, router_w, w12, w3):
    x = np.asarray(x, dtype=np.float32)
    ln_gamma = np.asarray(ln_gamma, dtype=np.float32)
    ln_beta = np.asarray(ln_beta, dtype=np.float32)
    router_w = np.asarray(router_w, dtype=np.float32)
    w12 = np.asarray(w12, dtype=np.float32)
    w3 = np.asarray(w3, dtype=np.float32)

    xn, experts, weights, tokens, pos, kept, slot = _route(
        x, ln_gamma, ln_beta, router_w)

    # dispatch: pack kept tokens into [E, C, D] (stable order, like the ref)
    buf = np.zeros((E * C + 1, D), np.float32)
    buf[slot] = xn[tokens]
    xb = buf[:E * C].reshape(E, C, D)

    bf = ml_dtypes.bfloat16
    in_maps = []
    for e in range(E):
        in_maps.append({
            "xbT": np.ascontiguousarray(xb[e].T).astype(bf),
            "w12T": np.ascontiguousarray(w12[e].T).astype(bf),
            "w3T": np.ascontiguousarray(w3[e].T).astype(bf),
        })

    if "nc" not in _CACHED:
        _CACHED["nc"] = build_nc()
    nc = _CACHED["nc"]

    import time as _time
    t0 = _time.time()
    res = run_bass_kernel_spmd(nc, in_maps, core_ids=list(range(E)))
    _CACHED["spmd_wall_s"] = _time.time() - t0

    yb = np.stack([np.asarray(res.results[e]["yT"], np.float32).T
                   for e in range(E)])          # [E, C, D]
    yb = yb.reshape(E * C, D)

    # combine: weight + scatter-add back to tokens
    ys = yb[np.minimum(slot, E * C - 1)] * (weights * kept)[:, None]
    out = np.zeros((T, D), np.float32)
    np.add.at(out, tokens, ys.astype(np.float32))
    return out.reshape(x.shape).astype(np.float32)
